# revision 31
# baseline (speedup 1.0000x reference)
"""Multi-head attention (B=2, L=2048, E=1024, H=16) on 8 trn2 NeuronCores.

Sharding: core c -> (batch b = c//4, head-group g = c%4). Each core computes
4 heads (256 feature dims) of one batch: Q/K/V projections column-sliced,
full attention for its heads, and its slice of the output projection
(Wo row-parallel). Host sums the 4 partial products per batch and adds
bo + Wo@bv (the bv term commutes through softmax-normalized attention).

v6 design notes (HW-measured 204us/rep vs v5's 256us on the same box):
 - symmetric 2-phase steady state: each For_i iteration runs two phases;
   phase s executes attention for set s while PE-filler quanta compute
   the OTHER set's output projection (yo) followed by its next-rep
   K/Q/V projections. Filler matmuls sit BETWEEN the score and PV
   matmuls of each k-tile step, so the PE never idles while ACT runs
   exp, and phases chain with no PE gap (no HAM re-throttle).
 - score matmuls for a head PAIR (partitions 0-63 / 64-127) issue
   back-to-back: contraction is 64 wide, so the two lower to disjoint
   PE row-groups (tile_position auto-derived from base_partition) and
   execute concurrently on the 128x128 array.
 - per-k-tile granularity, heads interleaved in one [128,2,512] PSUM
   tile: one full-tile exp per k-tile covers both heads (for diagonal
   tiles, cols 0:qo hold stale-but-finite garbage that PV never reads).
   PSUM: ps_att 2x2 + po 1x2 + fill 2x1 banks = 8.
 - partial-width APs spanning two PSUM banks crash the HW
   (NRT_EXEC_UNIT_UNRECOVERABLE): exp/reciprocal are emitted per-bank
   or full-tile only.
 - x streams as 48 [128,1024] half-chunks per phase in exactly the
   projection-pass consumption order (K, Q, then V; lp-major), so the
   chunk pool WAR-paces the DMA stream; K passes come first because
   the next phase's attention needs kT/qT dc=0 early and qT dc=1 last.
 - w/y DMAs issue from the otherwise-idle Pool (gpsimd) queue so the
   ACT sequencer is never blocked behind DMA descriptor generation.
 - softmax sum comes from a 64-wide ones block appended to V (output
   partitions are free on the PE); ones are memset ONCE in the
   preamble (projection copies only ever write v[:, :, :, 0:64]).
 - softmax without max-subtraction: scores are O(1) and masked entries
   are -1e9 -> exp underflows to exactly 0.
"""

import numpy as np

B, L, E, H = 2, 2048, 1024, 16
Dh = 64
NCORES = 8
HPC = 4           # heads per core
DG = HPC * Dh     # 256 features per core
NEG = np.float32(-1e9)

NE = E // 128     # 8 e-chunks
NQ = L // 512     # 4 q/ln blocks
NK = L // 128     # 16 k/l tiles

DT_MODE = "bf16"

_CACHE = {}
LAST_RESULTS = None


def _build(variant, dt_mode, reps=1, hw_loop=False, unroll_dual=False):
    import concourse.bacc as bacc
    import concourse.tile as tile
    from concourse import mybir
    from contextlib import ExitStack

    f32 = mybir.dt.float32
    bf16 = mybir.dt.bfloat16
    ddt = bf16          # dram dtype for x/w
    cdt = bf16          # sbuf compute dtype
    odt = bf16          # output store dtype

    dual = (hw_loop or unroll_dual) and reps > 1

    nc = bacc.Bacc()
    xq = nc.dram_tensor("xq", [128, NE, L], ddt, kind="ExternalInput")
    xk = nc.dram_tensor("xk", [128, NE, L], ddt, kind="ExternalInput")
    xv = nc.dram_tensor("xv", [128, NE, L], ddt, kind="ExternalInput")
    wq = nc.dram_tensor("wq", [128, NE, DG], ddt, kind="ExternalInput")
    wk = nc.dram_tensor("wk", [128, NE, DG], ddt, kind="ExternalInput")
    wv = nc.dram_tensor("wv", [128, NE, DG], ddt, kind="ExternalInput")
    bqs = nc.dram_tensor("bqs", [128, 2], f32, kind="ExternalInput")
    bks = nc.dram_tensor("bks", [128, 2], f32, kind="ExternalInput")
    wo = nc.dram_tensor("wo", [128, 2, E], ddt, kind="ExternalInput")
    dmask = maskT = None
    if variant == "causal":
        dmask = nc.dram_tensor("dmask", [128, 4, 512], f32, kind="ExternalInput")
    elif variant == "general":
        maskT = nc.dram_tensor("maskT", [L, L], f32, kind="ExternalInput")
    yD = nc.dram_tensor("yD", [128, NQ, NE, 512], odt, kind="ExternalOutput")

    Exp = mybir.ActivationFunctionType.Exp

    NSET = 2 if dual else 1
    # fill-quantum supply per rep: Q/K 64, V 32, yo 32
    NSTEP = (2 * sum(min(NK, 4 * qn + 4) for qn in range(NQ))
             if variant == "causal" else 2 * NQ * NK)

    with tile.TileContext(nc) as tc, ExitStack() as ctx:
        sb = ctx.enter_context(tc.tile_pool(name="sb", bufs=1))

        def settile(tag, shape, dt=None):
            return [sb.tile(shape, dt or cdt, tag=f"{tag}{s}",
                            name=f"{tag}{s}") for s in range(NSET)]

        qT_s = settile("qT", [128, 2, L])
        kT_s = settile("kT", [128, 2, L])
        v_s = settile("v", [128, NK, HPC, 128])
        st_s = settile("st", [128, 2, NQ, 512])
        wq_t = sb.tile([128, NE, DG], cdt, tag="wq", name="wq")
        wk_t = sb.tile([128, NE, DG], cdt, tag="wk", name="wk")
        wv_t = sb.tile([128, NE, DG], cdt, tag="wv", name="wv")
        wo_t = sb.tile([128, 2, E], cdt, tag="wo", name="wo")
        bq_t = sb.tile([128, 2], f32, tag="bq", name="bq")
        bk_t = sb.tile([128, 2], f32, tag="bk", name="bk")
        dm_t = None
        if variant == "causal":
            dm_t = sb.tile([128, 4, 512], f32, tag="dm", name="dm")

        XB = 26

        def w_dma():
            nc.gpsimd.dma_start(out=bq_t, in_=bqs[:, :])
            nc.gpsimd.dma_start(out=bk_t, in_=bks[:, :])
            nc.gpsimd.dma_start(out=wq_t, in_=wq[:, :, :])
            nc.gpsimd.dma_start(out=wk_t, in_=wk[:, :, :])
            nc.gpsimd.dma_start(out=wv_t, in_=wv[:, :, :])
            nc.gpsimd.dma_start(out=wo_t, in_=wo[:, :, :])
            if variant == "causal":
                nc.gpsimd.dma_start(out=dm_t, in_=dmask[:, :, :])

        def x_dma():
            # 48 half-chunk DMAs in projection-consumption order:
            # xk (lp0 ec0-7, lp1 ec0-7), xq same, xv (h0 ec0-7, h1 ec0-7)
            tiles = []
            for xT in (xk, xq, xv):
                for hf in range(2):
                    for ec in range(NE):
                        xt = sb.tile([128, 1, 1024], cdt, tag="xch",
                                     name="xch", bufs=XB)
                        nc.sync.dma_start(
                            out=xt,
                            in_=xT[:, ec:ec + 1, hf * 1024:(hf + 1) * 1024])
                        tiles.append(xt)
            return tiles

        def proj_quanta(s, xtiles, pp):
            # K before Q: what the next phase's attention needs LAST
            # (qT for p=1, V tail) is emitted last.
            for base, w_t, bias_t, scale, outT in (
                (0, wk_t, bk_t, 1.0, kT_s[s]),
                (16, wq_t, bq_t, 0.125, qT_s[s]),
            ):
                for lp in range(2):
                    for dc in range(2):
                        pj = [pp.tile([128, 512], mybir.dt.float32,
                                      tag=f"fq{i}", name=f"fq{i}")
                              for i in range(2)]
                        for ec in range(NE):
                            xt = xtiles[base + lp * NE + ec]
                            for li in range(2):
                                nc.tensor.matmul(
                                    out=pj[li],
                                    lhsT=w_t[:, ec, dc * 128:(dc + 1) * 128],
                                    rhs=xt[:, 0, li * 512:(li + 1) * 512],
                                    start=(ec == 0), stop=(ec == NE - 1),
                                )
                            yield
                        for li in range(2):
                            ln = 2 * lp + li
                            nc.vector.tensor_scalar(
                                out=outT[:, dc, ln * 512:(ln + 1) * 512],
                                in0=pj[li],
                                scalar1=scale,
                                scalar2=bias_t[:, dc:dc + 1],
                                op0=mybir.AluOpType.mult,
                                op1=mybir.AluOpType.add,
                            )
            # V passes: pa -> 4 l-tiles, 8 ec yields
            for pa in range(4):
                hf, off = pa // 2, (pa % 2) * 512
                pv = [pp.tile([128, 2, DG], mybir.dt.float32,
                              tag=f"fq{i}", name=f"fq{i}") for i in range(2)]
                for ec in range(NE):
                    xt = xtiles[32 + hf * NE + ec]
                    for lt4 in range(4):
                        nc.tensor.matmul(
                            out=pv[lt4 // 2][:, lt4 % 2, :],
                            lhsT=xt[:, 0, off + lt4 * 128:off + (lt4 + 1) * 128],
                            rhs=wv_t[:, ec, :],
                            start=(ec == 0 and lt4 % 2 == 0),
                            stop=(ec == NE - 1 and lt4 % 2 == 1),
                        )
                    yield
                for lt4 in range(4):
                    nc.vector.tensor_copy(
                        out=v_s[s][:, 4 * pa + lt4, :, 0:Dh],
                        in_=pv[lt4 // 2][:, lt4 % 2, :]
                        .rearrange("p (h d) -> p h d", h=HPC),
                    )

        def yo_quanta(s, fp):
            # out-projection of set s as 2-bank quanta; yield per jc
            for ln in range(NQ):
                yt = sb.tile([128, NE, 512], odt, tag="yt", name="yt",
                             bufs=2)
                for jc in range(NE):
                    yps = fp.tile([128, 512], mybir.dt.float32,
                                  tag=f"fq{jc % 2}", name=f"fq{jc % 2}")
                    for dc in range(2):
                        nc.tensor.matmul(
                            out=yps,
                            lhsT=wo_t[:, dc, jc * 128:(jc + 1) * 128],
                            rhs=st_s[s][:, dc, ln, :],
                            start=(dc == 0), stop=(dc == 1),
                        )
                    nc.vector.tensor_copy(out=yt[:, jc, :], in_=yps)
                    yield
                nc.gpsimd.dma_start(out=yD[:, ln, :, :], in_=yt)

        def memset_ones():
            for s in range(NSET):
                nc.vector.memset(v_s[s][:, :, :, Dh:128], 1.0)

        def attn_out(s, fill=None, nyield=128):
            from contextlib import nullcontext
            fcm = (tc.tile_pool(name="ps_fill", bufs=1, space="PSUM")
                   if fill else nullcontext())
            with tc.tile_pool(name="ps_att", bufs=2, space="PSUM") as sp, \
                 tc.tile_pool(name="ps_out", bufs=1, space="PSUM") as op, \
                 fcm as fp:
                gen = iter(fill(fp)) if fill else None
                taken, steps_done = 0, 0
                for p in range(2):
                    dc = p
                    for qn in range(NQ):
                        kmax = (min(NK, 4 * qn + 4)
                                if variant == "causal" else NK)
                        po = op.tile([128, 2, 512], mybir.dt.float32,
                                     tag="po", name="po")
                        for kc in range(kmax):
                            db = kc - 4 * qn
                            diag = variant == "causal" and db >= 0
                            qo = db * 128 if diag else 0
                            ps = sp.tile([128, 2, 512], mybir.dt.float32,
                                         tag="pss", name="pss")
                            for hh in range(2):
                                nc.tensor.matmul(
                                    out=ps[:, hh, qo:512],
                                    lhsT=kT_s[s][64 * hh:64 * hh + 64, dc,
                                                 kc * 128:(kc + 1) * 128],
                                    rhs=qT_s[s][64 * hh:64 * hh + 64, dc,
                                                qn * 512 + qo:(qn + 1) * 512],
                                    start=True, stop=True,
                                )
                            mkt = None
                            if variant == "general":
                                mkt = sb.tile([128, 512], mybir.dt.float32,
                                              tag="mkt", name="mkt", bufs=3)
                                nc.sync.dma_start(
                                    out=mkt,
                                    in_=maskT[kc * 128:(kc + 1) * 128,
                                              qn * 512:(qn + 1) * 512])
                                for hh in range(2):
                                    nc.vector.tensor_add(
                                        out=ps[:, hh, :], in0=ps[:, hh, :],
                                        in1=mkt)
                            elif diag:
                                for hh in range(2):
                                    nc.vector.tensor_add(
                                        out=ps[:, hh, qo:qo + 128],
                                        in0=ps[:, hh, qo:qo + 128],
                                        in1=dm_t[:, db, qo:qo + 128])
                            pt = sb.tile([128, 2, 512], cdt, tag="pt",
                                         name="pt", bufs=8)
                            # full contiguous tile (HW-proven AP): for
                            # diag tiles cols 0:qo hold stale scores whose
                            # exp is finite garbage that PV never reads
                            nc.scalar.activation(out=pt, in_=ps, func=Exp)
                            if gen is not None:
                                steps_done += 1
                                want = (nyield * steps_done) // NSTEP
                                while taken < want and \
                                        next(gen, "END") != "END":
                                    taken += 1
                            for hh in range(2):
                                nc.tensor.matmul(
                                    out=po[:, hh, qo:512],
                                    lhsT=v_s[s][:, kc, 2 * p + hh, :],
                                    rhs=pt[:, hh, qo:512],
                                    start=(kc == 0),
                                    stop=(kc == kmax - 1),
                                )
                        rec = sb.tile([64, 2, 512], mybir.dt.float32,
                                      tag="rec", name="rec", bufs=4)
                        for hh in range(2):
                            nc.vector.reciprocal(out=rec[:, hh, :],
                                                 in_=po[64:128, hh, :])
                        for hh in range(2):
                            nc.vector.tensor_mul(
                                out=st_s[s][64 * hh:64 * hh + Dh, dc, qn, :],
                                in0=po[0:Dh, hh, :], in1=rec[:, hh, :])
                if gen is not None:
                    for _ in gen:
                        pass

        def drain(gen_fn):
            with tc.tile_pool(name="ps_drain", bufs=1, space="PSUM") as fp:
                for _ in gen_fn(fp):
                    pass

        def preamble():
            w_dma()
            memset_ones()
            xt0 = x_dma()
            drain(lambda fp: proj_quanta(0, xt0, fp))
            if dual:
                nc.vector.memset(st_s[1][:, :, :, :], 0.0)

        def phase(s):
            xt = x_dma()
            s1 = 1 - s

            def fill(fp):
                yield from yo_quanta(s1, fp)
                yield from proj_quanta(s1, xt, fp)
            attn_out(s, fill=fill, nyield=128)

        if dual and not unroll_dual:
            preamble()
            if reps >= 8:
                with tc.For_i(0, reps // 4, 1, hint_engines=(
                        mybir.EngineType.PE,)):
                    phase(0)
                    phase(1)
                    phase(0)
                    phase(1)
            else:
                with tc.For_i(0, reps // 2, 1, hint_engines=(
                        mybir.EngineType.PE,)):
                    phase(0)
                    phase(1)
            drain(lambda fp: yo_quanta(1, fp))
        elif dual:
            preamble()
            for _ in range(reps // 2):
                phase(0)
                phase(1)
            drain(lambda fp: yo_quanta(1, fp))
        else:
            for _ in range(reps):
                preamble()
                attn_out(0)
                drain(lambda fp: yo_quanta(0, fp))

    nc.finalize()
    return nc


def _get_nc(variant, dt_mode=None, reps=1, hw_loop=False, unroll_dual=False):
    dt_mode = dt_mode or DT_MODE
    key = (variant, dt_mode, reps, hw_loop, unroll_dual)
    if key not in _CACHE:
        _CACHE[key] = _build(variant, dt_mode, reps, hw_loop, unroll_dual)
    return _CACHE[key]


def _detect_variant(mask):
    m2 = np.asarray(mask).reshape(mask.shape[-2], mask.shape[-1])
    m01 = (m2 != 0)
    if m01.all():
        return "none", m2
    if np.array_equal(m01, np.tril(np.ones(m2.shape, bool))):
        return "causal", m2
    return "general", m2


def _dmask_np():
    kl = np.arange(128)[:, None, None]
    db = np.arange(4)[None, :, None]
    ql = np.arange(512)[None, None, :]
    return np.where(db * 128 + kl > ql, NEG, np.float32(0)).astype(np.float32)


def _pack_pm(aT):
    # [R, C] with R = NE*128 -> [128, NE, C] partition-major
    r, c = aT.shape
    return np.ascontiguousarray(
        aT.reshape(r // 128, 128, c).transpose(1, 0, 2))


def _cvt(a, dt_mode):
    import ml_dtypes
    return np.ascontiguousarray(a.astype(ml_dtypes.bfloat16))


def _make_in_maps(x_q, x_k, x_v, m2, variant, Wq, bq, Wk, bk, Wv, Wo,
                  dt_mode=None):
    dt_mode = dt_mode or DT_MODE
    in_maps = []
    madd = None
    if variant == "general":
        madd = np.ascontiguousarray(
            np.where(m2 == 0, NEG, np.float32(0)).astype(np.float32).T)
    dmn = _dmask_np() if variant == "causal" else None
    xs = {b: tuple(_cvt(_pack_pm(x[b].T), dt_mode)
                   for x in (x_q, x_k, x_v)) for b in range(B)}
    for c in range(NCORES):
        b, g = divmod(c, HPC)
        gs = slice(g * DG, (g + 1) * DG)
        # wo2[(h%2)*64 + d, h//2, :] = Wo[:, g*DG + h*64 + d]
        wog = Wo[:, gs].T.reshape(HPC, Dh, E)
        wo2 = np.zeros((128, 2, E), np.float32)
        for h in range(HPC):
            wo2[(h % 2) * Dh:(h % 2 + 1) * Dh, h // 2, :] = wog[h]
        im = {
            "xq": xs[b][0],
            "xk": xs[b][1],
            "xv": xs[b][2],
            "wq": _cvt(_pack_pm(Wq[gs, :].T), dt_mode),
            "wk": _cvt(_pack_pm(Wk[gs, :].T), dt_mode),
            "wv": _cvt(_pack_pm(Wv[gs, :].T), dt_mode),
            "bqs": np.ascontiguousarray((bq[gs] / 8.0).reshape(2, 128).T),
            "bks": np.ascontiguousarray(bk[gs].reshape(2, 128).T),
            "wo": _cvt(wo2, dt_mode),
        }
        if variant == "causal":
            im["dmask"] = dmn
        elif variant == "general":
            im["maskT"] = madd
        in_maps.append(im)
    return in_maps


def kernel(x_q, x_k, x_v, mask, Wq, bq, Wk, bk, Wv, bv, Wo, bo):
    global LAST_RESULTS
    from concourse.bass_utils import run_bass_kernel_spmd

    x_q = np.asarray(x_q, np.float32)
    x_k = np.asarray(x_k, np.float32)
    x_v = np.asarray(x_v, np.float32)
    Wq = np.asarray(Wq, np.float32)
    Wk = np.asarray(Wk, np.float32)
    Wv = np.asarray(Wv, np.float32)
    Wo = np.asarray(Wo, np.float32)
    bq = np.asarray(bq, np.float32)
    bk = np.asarray(bk, np.float32)
    bv = np.asarray(bv, np.float32)
    bo = np.asarray(bo, np.float32)

    variant, m2 = _detect_variant(mask)
    nc = _get_nc(variant)
    in_maps = _make_in_maps(x_q, x_k, x_v, m2, variant, Wq, bq, Wk, bk, Wv, Wo)

    res = run_bass_kernel_spmd(nc, in_maps, core_ids=list(range(NCORES)))
    LAST_RESULTS = res

    corr = (bo + Wo @ bv).astype(np.float32)
    y = np.empty((B, L, E), np.float32)
    for b in range(B):
        acc = res.results[HPC * b]["yD"].astype(np.float32)
        for g in range(1, HPC):
            acc += res.results[HPC * b + g]["yD"].astype(np.float32)
        # [128(p), NQ(ln), NE(jc), 512(q)] -> [L, E]
        y[b] = acc.transpose(1, 3, 2, 0).reshape(L, E) + corr
    return y


# revision 32
# speedup vs baseline: 1.1052x; 1.1052x over previous
"""Multi-head attention (B=2, L=2048, E=1024, H=16) on 8 trn2 NeuronCores.

Sharding: core c -> (batch b = c//4, head-group g = c%4). Each core computes
4 heads (256 feature dims) of one batch: Q/K/V projections column-sliced,
full attention for its heads, and its slice of the output projection
(Wo row-parallel). Host sums the 4 partial products per batch and adds
bo + Wo@bv (the bv term commutes through softmax-normalized attention).

v6 design notes (HW-measured 204us/rep vs v5's 256us on the same box):
 - symmetric 2-phase steady state: each For_i iteration runs two phases;
   phase s executes attention for set s while PE-filler quanta compute
   the OTHER set's output projection (yo) followed by its next-rep
   K/Q/V projections. Filler matmuls sit BETWEEN the score and PV
   matmuls of each k-tile step, so the PE never idles while ACT runs
   exp, and phases chain with no PE gap (no HAM re-throttle).
 - score matmuls for a head PAIR (partitions 0-63 / 64-127) issue
   back-to-back: contraction is 64 wide, so the two lower to disjoint
   PE row-groups (tile_position auto-derived from base_partition) and
   execute concurrently on the 128x128 array.
 - per-k-tile granularity, heads interleaved in one [128,2,512] PSUM
   tile: one full-tile exp per k-tile covers both heads (for diagonal
   tiles, cols 0:qo hold stale-but-finite garbage that PV never reads).
   PSUM: ps_att 2x2 + po 1x2 + fill 2x1 banks = 8.
 - partial-width APs spanning two PSUM banks crash the HW
   (NRT_EXEC_UNIT_UNRECOVERABLE): exp/reciprocal are emitted per-bank
   or full-tile only.
 - x streams as 48 [128,1024] half-chunks per phase in exactly the
   projection-pass consumption order (K, Q, then V; lp-major), so the
   chunk pool WAR-paces the DMA stream; K passes come first because
   the next phase's attention needs kT/qT dc=0 early and qT dc=1 last.
 - w/y DMAs issue from the otherwise-idle Pool (gpsimd) queue so the
   ACT sequencer is never blocked behind DMA descriptor generation.
 - softmax sum comes from a 64-wide ones block appended to V (output
   partitions are free on the PE); ones are memset ONCE in the
   preamble (projection copies only ever write v[:, :, :, 0:64]).
 - softmax without max-subtraction: scores are O(1) and masked entries
   are -1e9 -> exp underflows to exactly 0.
"""

import numpy as np

B, L, E, H = 2, 2048, 1024, 16
Dh = 64
NCORES = 8
HPC = 4           # heads per core
DG = HPC * Dh     # 256 features per core
NEG = np.float32(-1e9)

NE = E // 128     # 8 e-chunks
NQ = L // 512     # 4 q/ln blocks
NK = L // 128     # 16 k/l tiles

DT_MODE = "bf16"

_CACHE = {}
LAST_RESULTS = None


def _build(variant, dt_mode, reps=1, hw_loop=False, unroll_dual=False):
    import concourse.bacc as bacc
    import concourse.tile as tile
    from concourse import mybir
    from contextlib import ExitStack

    f32 = mybir.dt.float32
    bf16 = mybir.dt.bfloat16
    ddt = bf16          # dram dtype for x/w
    cdt = bf16          # sbuf compute dtype
    odt = bf16          # output store dtype

    dual = (hw_loop or unroll_dual) and reps > 1

    nc = bacc.Bacc()
    xq = nc.dram_tensor("xq", [128, NE, L], ddt, kind="ExternalInput")
    xk = nc.dram_tensor("xk", [128, NE, L], ddt, kind="ExternalInput")
    xv = nc.dram_tensor("xv", [128, NE, L], ddt, kind="ExternalInput")
    wq = nc.dram_tensor("wq", [128, NE, DG], ddt, kind="ExternalInput")
    wk = nc.dram_tensor("wk", [128, NE, DG], ddt, kind="ExternalInput")
    wv = nc.dram_tensor("wv", [128, NE, DG], ddt, kind="ExternalInput")
    bqs = nc.dram_tensor("bqs", [128, 2], f32, kind="ExternalInput")
    bks = nc.dram_tensor("bks", [128, 2], f32, kind="ExternalInput")
    wo = nc.dram_tensor("wo", [128, 2, E], ddt, kind="ExternalInput")
    dmask = maskT = None
    if variant == "causal":
        dmask = nc.dram_tensor("dmask", [128, 4, 512], f32, kind="ExternalInput")
    elif variant == "general":
        maskT = nc.dram_tensor("maskT", [L, L], f32, kind="ExternalInput")
    yD = nc.dram_tensor("yD", [128, NQ, NE, 512], odt, kind="ExternalOutput")

    Exp = mybir.ActivationFunctionType.Exp

    NSET = 2 if dual else 1
    # fill-quantum supply per rep: Q/K 64, V 32, yo 32
    NSTEP = (2 * sum(min(NK, 4 * qn + 4) for qn in range(NQ))
             if variant == "causal" else 2 * NQ * NK)

    with tile.TileContext(nc) as tc, ExitStack() as ctx:
        sb = ctx.enter_context(tc.tile_pool(name="sb", bufs=1))

        def settile(tag, shape, dt=None):
            return [sb.tile(shape, dt or cdt, tag=f"{tag}{s}",
                            name=f"{tag}{s}") for s in range(NSET)]

        qT_s = settile("qT", [128, 2, L])
        kT_s = settile("kT", [128, 2, L])
        v_s = settile("v", [128, NK, HPC, 128])
        st_s = settile("st", [128, 2, NQ, 512])
        wq_t = sb.tile([128, NE, DG], cdt, tag="wq", name="wq")
        wk_t = sb.tile([128, NE, DG], cdt, tag="wk", name="wk")
        wv_t = sb.tile([128, NE, DG], cdt, tag="wv", name="wv")
        wo_t = sb.tile([128, 2, E], cdt, tag="wo", name="wo")
        bq_t = sb.tile([128, 2], f32, tag="bq", name="bq")
        bk_t = sb.tile([128, 2], f32, tag="bk", name="bk")
        dm_t = None
        if variant == "causal":
            dm_t = sb.tile([128, 4, 512], f32, tag="dm", name="dm")

        XB = 22

        def w_dma():
            nc.gpsimd.dma_start(out=bq_t, in_=bqs[:, :])
            nc.gpsimd.dma_start(out=bk_t, in_=bks[:, :])
            nc.gpsimd.dma_start(out=wq_t, in_=wq[:, :, :])
            nc.gpsimd.dma_start(out=wk_t, in_=wk[:, :, :])
            nc.gpsimd.dma_start(out=wv_t, in_=wv[:, :, :])
            nc.gpsimd.dma_start(out=wo_t, in_=wo[:, :, :])
            if variant == "causal":
                nc.gpsimd.dma_start(out=dm_t, in_=dmask[:, :, :])

        def x_dma():
            # 48 half-chunk DMAs in projection-consumption order:
            # xk (lp0 ec0-7, lp1 ec0-7), xq same, xv (h0 ec0-7, h1 ec0-7)
            tiles = []
            for xT in (xk, xq, xv):
                for hf in range(2):
                    for ec in range(NE):
                        xt = sb.tile([128, 1, 1024], cdt, tag="xch",
                                     name="xch", bufs=XB)
                        nc.sync.dma_start(
                            out=xt,
                            in_=xT[:, ec:ec + 1, hf * 1024:(hf + 1) * 1024])
                        tiles.append(xt)
            return tiles

        def proj_quanta(s, xtiles, pp):
            # K before Q: what the next phase's attention needs LAST
            # (qT for p=1, V tail) is emitted last.
            for base, w_t, bias_t, scale, outT in (
                (0, wk_t, bk_t, 1.0, kT_s[s]),
                (16, wq_t, bq_t, 0.125, qT_s[s]),
            ):
                for lp in range(2):
                    for dc in range(2):
                        pj = [pp.tile([128, 512], mybir.dt.float32,
                                      tag=f"fq{i}", name=f"fq{i}")
                              for i in range(2)]
                        for ec in range(NE):
                            xt = xtiles[base + lp * NE + ec]
                            for li in range(2):
                                nc.tensor.matmul(
                                    out=pj[li],
                                    lhsT=w_t[:, ec, dc * 128:(dc + 1) * 128],
                                    rhs=xt[:, 0, li * 512:(li + 1) * 512],
                                    start=(ec == 0), stop=(ec == NE - 1),
                                )
                            yield
                        for li in range(2):
                            ln = 2 * lp + li
                            nc.vector.tensor_scalar(
                                out=outT[:, dc, ln * 512:(ln + 1) * 512],
                                in0=pj[li],
                                scalar1=scale,
                                scalar2=bias_t[:, dc:dc + 1],
                                op0=mybir.AluOpType.mult,
                                op1=mybir.AluOpType.add,
                            )
            # V passes: pa -> 4 l-tiles, 8 ec yields
            for pa in range(4):
                hf, off = pa // 2, (pa % 2) * 512
                pv = [pp.tile([128, 2, DG], mybir.dt.float32,
                              tag=f"fq{i}", name=f"fq{i}") for i in range(2)]
                for ec in range(NE):
                    xt = xtiles[32 + hf * NE + ec]
                    for lt4 in range(4):
                        nc.tensor.matmul(
                            out=pv[lt4 // 2][:, lt4 % 2, :],
                            lhsT=xt[:, 0, off + lt4 * 128:off + (lt4 + 1) * 128],
                            rhs=wv_t[:, ec, :],
                            start=(ec == 0 and lt4 % 2 == 0),
                            stop=(ec == NE - 1 and lt4 % 2 == 1),
                        )
                    yield
                for lt4 in range(4):
                    nc.vector.tensor_copy(
                        out=v_s[s][:, 4 * pa + lt4, :, 0:Dh],
                        in_=pv[lt4 // 2][:, lt4 % 2, :]
                        .rearrange("p (h d) -> p h d", h=HPC),
                    )

        def yo_quanta(s, fp):
            # out-projection of set s as 2-bank quanta; yield per jc
            for ln in range(NQ):
                yt = sb.tile([128, NE, 512], odt, tag="yt", name="yt",
                             bufs=2)
                for jc in range(NE):
                    yps = fp.tile([128, 512], mybir.dt.float32,
                                  tag=f"fq{jc % 2}", name=f"fq{jc % 2}")
                    for dc in range(2):
                        nc.tensor.matmul(
                            out=yps,
                            lhsT=wo_t[:, dc, jc * 128:(jc + 1) * 128],
                            rhs=st_s[s][:, dc, ln, :],
                            start=(dc == 0), stop=(dc == 1),
                        )
                    nc.vector.tensor_copy(out=yt[:, jc, :], in_=yps)
                    yield
                nc.gpsimd.dma_start(out=yD[:, ln, :, :], in_=yt)

        def memset_ones():
            for s in range(NSET):
                nc.vector.memset(v_s[s][:, :, :, Dh:128], 1.0)

        def attn_out(s, fill=None, nyield=128):
            from contextlib import nullcontext
            fcm = (tc.tile_pool(name="ps_fill", bufs=1, space="PSUM")
                   if fill else nullcontext())
            with tc.tile_pool(name="ps_att", bufs=2, space="PSUM") as sp, \
                 tc.tile_pool(name="ps_out", bufs=1, space="PSUM") as op, \
                 fcm as fp:
                gen = iter(fill(fp)) if fill else None
                taken, steps_done = 0, 0
                for p in range(2):
                    dc = p
                    for qn in range(NQ):
                        kmax = (min(NK, 4 * qn + 4)
                                if variant == "causal" else NK)
                        po = op.tile([128, 2, 512], mybir.dt.float32,
                                     tag="po", name="po")
                        for kc in range(kmax):
                            db = kc - 4 * qn
                            diag = variant == "causal" and db >= 0
                            qo = db * 128 if diag else 0
                            ps = sp.tile([128, 2, 512], mybir.dt.float32,
                                         tag="pss", name="pss")
                            for hh in range(2):
                                nc.tensor.matmul(
                                    out=ps[:, hh, qo:512],
                                    lhsT=kT_s[s][64 * hh:64 * hh + 64, dc,
                                                 kc * 128:(kc + 1) * 128],
                                    rhs=qT_s[s][64 * hh:64 * hh + 64, dc,
                                                qn * 512 + qo:(qn + 1) * 512],
                                    start=True, stop=True,
                                )
                            mkt = None
                            if variant == "general":
                                mkt = sb.tile([128, 512], mybir.dt.float32,
                                              tag="mkt", name="mkt", bufs=3)
                                nc.sync.dma_start(
                                    out=mkt,
                                    in_=maskT[kc * 128:(kc + 1) * 128,
                                              qn * 512:(qn + 1) * 512])
                                for hh in range(2):
                                    nc.vector.tensor_add(
                                        out=ps[:, hh, :], in0=ps[:, hh, :],
                                        in1=mkt)
                            elif diag:
                                for hh in range(2):
                                    nc.vector.tensor_add(
                                        out=ps[:, hh, qo:qo + 128],
                                        in0=ps[:, hh, qo:qo + 128],
                                        in1=dm_t[:, db, qo:qo + 128])
                            pt = sb.tile([128, 2, 512], cdt, tag="pt",
                                         name="pt", bufs=6)
                            # full contiguous tile (HW-proven AP): for
                            # diag tiles cols 0:qo hold stale scores whose
                            # exp is finite garbage that PV never reads
                            nc.scalar.activation(out=pt, in_=ps, func=Exp)
                            if gen is not None:
                                steps_done += 1
                                want = (nyield * steps_done) // NSTEP
                                while taken < want and \
                                        next(gen, "END") != "END":
                                    taken += 1
                            for hh in range(2):
                                nc.tensor.matmul(
                                    out=po[:, hh, qo:512],
                                    lhsT=v_s[s][:, kc, 2 * p + hh, :],
                                    rhs=pt[:, hh, qo:512],
                                    start=(kc == 0),
                                    stop=(kc == kmax - 1),
                                )
                        rec = sb.tile([64, 2, 512], mybir.dt.float32,
                                      tag="rec", name="rec", bufs=2)
                        for hh in range(2):
                            nc.vector.reciprocal(out=rec[:, hh, :],
                                                 in_=po[64:128, hh, :])
                        for hh in range(2):
                            nc.vector.tensor_mul(
                                out=st_s[s][64 * hh:64 * hh + Dh, dc, qn, :],
                                in0=po[0:Dh, hh, :], in1=rec[:, hh, :])
                if gen is not None:
                    for _ in gen:
                        pass

        def drain(gen_fn):
            with tc.tile_pool(name="ps_drain", bufs=1, space="PSUM") as fp:
                for _ in gen_fn(fp):
                    pass

        def preamble():
            w_dma()
            memset_ones()
            xt0 = x_dma()
            drain(lambda fp: proj_quanta(0, xt0, fp))
            if dual:
                nc.vector.memset(st_s[1][:, :, :, :], 0.0)

        def phase(s):
            xt = x_dma()
            s1 = 1 - s

            def fill(fp):
                yield from yo_quanta(s1, fp)
                yield from proj_quanta(s1, xt, fp)
            attn_out(s, fill=fill, nyield=128)

        if dual and not unroll_dual:
            preamble()
            if reps >= 8:
                with tc.For_i(0, reps // 4, 1, hint_engines=(
                        mybir.EngineType.PE,)):
                    phase(0)
                    phase(1)
                    phase(0)
                    phase(1)
            else:
                with tc.For_i(0, reps // 2, 1, hint_engines=(
                        mybir.EngineType.PE,)):
                    phase(0)
                    phase(1)
            drain(lambda fp: yo_quanta(1, fp))
        elif dual:
            preamble()
            for _ in range(reps // 2):
                phase(0)
                phase(1)
            drain(lambda fp: yo_quanta(1, fp))
        else:
            for _ in range(reps):
                preamble()
                attn_out(0)
                drain(lambda fp: yo_quanta(0, fp))

    nc.finalize()
    return nc


def _get_nc(variant, dt_mode=None, reps=1, hw_loop=False, unroll_dual=False):
    dt_mode = dt_mode or DT_MODE
    key = (variant, dt_mode, reps, hw_loop, unroll_dual)
    if key not in _CACHE:
        _CACHE[key] = _build(variant, dt_mode, reps, hw_loop, unroll_dual)
    return _CACHE[key]


def _detect_variant(mask):
    m2 = np.asarray(mask).reshape(mask.shape[-2], mask.shape[-1])
    m01 = (m2 != 0)
    if m01.all():
        return "none", m2
    if np.array_equal(m01, np.tril(np.ones(m2.shape, bool))):
        return "causal", m2
    return "general", m2


def _dmask_np():
    kl = np.arange(128)[:, None, None]
    db = np.arange(4)[None, :, None]
    ql = np.arange(512)[None, None, :]
    return np.where(db * 128 + kl > ql, NEG, np.float32(0)).astype(np.float32)


def _pack_pm(aT):
    # [R, C] with R = NE*128 -> [128, NE, C] partition-major
    r, c = aT.shape
    return np.ascontiguousarray(
        aT.reshape(r // 128, 128, c).transpose(1, 0, 2))


def _cvt(a, dt_mode):
    import ml_dtypes
    return np.ascontiguousarray(a.astype(ml_dtypes.bfloat16))


def _make_in_maps(x_q, x_k, x_v, m2, variant, Wq, bq, Wk, bk, Wv, Wo,
                  dt_mode=None):
    dt_mode = dt_mode or DT_MODE
    in_maps = []
    madd = None
    if variant == "general":
        madd = np.ascontiguousarray(
            np.where(m2 == 0, NEG, np.float32(0)).astype(np.float32).T)
    dmn = _dmask_np() if variant == "causal" else None
    xs = {b: tuple(_cvt(_pack_pm(x[b].T), dt_mode)
                   for x in (x_q, x_k, x_v)) for b in range(B)}
    for c in range(NCORES):
        b, g = divmod(c, HPC)
        gs = slice(g * DG, (g + 1) * DG)
        # wo2[(h%2)*64 + d, h//2, :] = Wo[:, g*DG + h*64 + d]
        wog = Wo[:, gs].T.reshape(HPC, Dh, E)
        wo2 = np.zeros((128, 2, E), np.float32)
        for h in range(HPC):
            wo2[(h % 2) * Dh:(h % 2 + 1) * Dh, h // 2, :] = wog[h]
        im = {
            "xq": xs[b][0],
            "xk": xs[b][1],
            "xv": xs[b][2],
            "wq": _cvt(_pack_pm(Wq[gs, :].T), dt_mode),
            "wk": _cvt(_pack_pm(Wk[gs, :].T), dt_mode),
            "wv": _cvt(_pack_pm(Wv[gs, :].T), dt_mode),
            "bqs": np.ascontiguousarray((bq[gs] / 8.0).reshape(2, 128).T),
            "bks": np.ascontiguousarray(bk[gs].reshape(2, 128).T),
            "wo": _cvt(wo2, dt_mode),
        }
        if variant == "causal":
            im["dmask"] = dmn
        elif variant == "general":
            im["maskT"] = madd
        in_maps.append(im)
    return in_maps


def kernel(x_q, x_k, x_v, mask, Wq, bq, Wk, bk, Wv, bv, Wo, bo):
    global LAST_RESULTS
    from concourse.bass_utils import run_bass_kernel_spmd

    x_q = np.asarray(x_q, np.float32)
    x_k = np.asarray(x_k, np.float32)
    x_v = np.asarray(x_v, np.float32)
    Wq = np.asarray(Wq, np.float32)
    Wk = np.asarray(Wk, np.float32)
    Wv = np.asarray(Wv, np.float32)
    Wo = np.asarray(Wo, np.float32)
    bq = np.asarray(bq, np.float32)
    bk = np.asarray(bk, np.float32)
    bv = np.asarray(bv, np.float32)
    bo = np.asarray(bo, np.float32)

    variant, m2 = _detect_variant(mask)
    nc = _get_nc(variant)
    in_maps = _make_in_maps(x_q, x_k, x_v, m2, variant, Wq, bq, Wk, bk, Wv, Wo)

    res = run_bass_kernel_spmd(nc, in_maps, core_ids=list(range(NCORES)))
    LAST_RESULTS = res

    corr = (bo + Wo @ bv).astype(np.float32)
    y = np.empty((B, L, E), np.float32)
    for b in range(B):
        acc = res.results[HPC * b]["yD"].astype(np.float32)
        for g in range(1, HPC):
            acc += res.results[HPC * b + g]["yD"].astype(np.float32)
        # [128(p), NQ(ln), NE(jc), 512(q)] -> [L, E]
        y[b] = acc.transpose(1, 3, 2, 0).reshape(L, E) + corr
    return y


# revision 34
# speedup vs baseline: 1.1404x; 1.0319x over previous
"""Multi-head attention (B=2, L=2048, E=1024, H=16) on 8 trn2 NeuronCores.

Sharding: core c -> (batch b = c//4, head-group g = c%4). Each core computes
4 heads (256 feature dims) of one batch: Q/K/V projections column-sliced,
full attention for its heads, and its slice of the output projection
(Wo row-parallel). Host sums the 4 partial products per batch and adds
bo + Wo@bv (the bv term commutes through softmax-normalized attention).

v6 design notes (HW-measured 204us/rep vs v5's 256us on the same box):
 - symmetric 2-phase steady state: each For_i iteration runs two phases;
   phase s executes attention for set s while PE-filler quanta compute
   the OTHER set's output projection (yo) followed by its next-rep
   K/Q/V projections. Filler matmuls sit BETWEEN the score and PV
   matmuls of each k-tile step, so the PE never idles while ACT runs
   exp, and phases chain with no PE gap (no HAM re-throttle).
 - score matmuls for a head PAIR (partitions 0-63 / 64-127) issue
   back-to-back: contraction is 64 wide, so the two lower to disjoint
   PE row-groups (tile_position auto-derived from base_partition) and
   execute concurrently on the 128x128 array.
 - per-k-tile granularity, heads interleaved in one [128,2,512] PSUM
   tile: one full-tile exp per k-tile covers both heads (for diagonal
   tiles, cols 0:qo hold stale-but-finite garbage that PV never reads).
   PSUM: ps_att 2x2 + po 1x2 + fill 2x1 banks = 8.
 - partial-width APs spanning two PSUM banks crash the HW
   (NRT_EXEC_UNIT_UNRECOVERABLE): exp/reciprocal are emitted per-bank
   or full-tile only.
 - x streams as 48 [128,1024] half-chunks per phase in exactly the
   projection-pass consumption order (K, Q, then V; lp-major), so the
   chunk pool WAR-paces the DMA stream; K passes come first because
   the next phase's attention needs kT/qT dc=0 early and qT dc=1 last.
 - w/y DMAs issue from the otherwise-idle Pool (gpsimd) queue so the
   ACT sequencer is never blocked behind DMA descriptor generation.
 - softmax sum comes from a 64-wide ones block appended to V (output
   partitions are free on the PE); ones are memset ONCE in the
   preamble (projection copies only ever write v[:, :, :, 0:64]).
 - softmax without max-subtraction: scores are O(1) and masked entries
   are -1e9 -> exp underflows to exactly 0.
"""

import numpy as np

B, L, E, H = 2, 2048, 1024, 16
Dh = 64
NCORES = 8
HPC = 4           # heads per core
DG = HPC * Dh     # 256 features per core
NEG = np.float32(-1e9)

NE = E // 128     # 8 e-chunks
NQ = L // 512     # 4 q/ln blocks
NK = L // 128     # 16 k/l tiles

DT_MODE = "bf16"

_CACHE = {}
LAST_RESULTS = None


def _build(variant, dt_mode, reps=1, hw_loop=False, unroll_dual=False):
    import concourse.bacc as bacc
    import concourse.tile as tile
    from concourse import mybir
    from contextlib import ExitStack

    f32 = mybir.dt.float32
    bf16 = mybir.dt.bfloat16
    ddt = bf16          # dram dtype for x/w
    cdt = bf16          # sbuf compute dtype
    odt = bf16          # output store dtype

    dual = (hw_loop or unroll_dual) and reps > 1

    nc = bacc.Bacc()
    xq = nc.dram_tensor("xq", [128, NE, L], ddt, kind="ExternalInput")
    xk = nc.dram_tensor("xk", [128, NE, L], ddt, kind="ExternalInput")
    xv = nc.dram_tensor("xv", [128, NE, L], ddt, kind="ExternalInput")
    wq = nc.dram_tensor("wq", [128, NE, DG], ddt, kind="ExternalInput")
    wk = nc.dram_tensor("wk", [128, NE, DG], ddt, kind="ExternalInput")
    wv = nc.dram_tensor("wv", [128, NE, DG], ddt, kind="ExternalInput")
    bqs = nc.dram_tensor("bqs", [128, 2], f32, kind="ExternalInput")
    bks = nc.dram_tensor("bks", [128, 2], f32, kind="ExternalInput")
    wo = nc.dram_tensor("wo", [128, 2, E], ddt, kind="ExternalInput")
    dmask = maskT = None
    if variant == "causal":
        dmask = nc.dram_tensor("dmask", [128, 4, 512], f32, kind="ExternalInput")
    elif variant == "general":
        maskT = nc.dram_tensor("maskT", [L, L], f32, kind="ExternalInput")
    yD = nc.dram_tensor("yD", [128, NQ, NE, 512], odt, kind="ExternalOutput")

    Exp = mybir.ActivationFunctionType.Exp

    NSET = 2 if dual else 1
    # fill-quantum supply per rep: Q/K 64, V 32, yo 32
    NSTEP = (2 * sum(min(NK, 4 * qn + 4) for qn in range(NQ))
             if variant == "causal" else 2 * NQ * NK)

    with tile.TileContext(nc) as tc, ExitStack() as ctx:
        sb = ctx.enter_context(tc.tile_pool(name="sb", bufs=1))

        def settile(tag, shape, dt=None):
            return [sb.tile(shape, dt or cdt, tag=f"{tag}{s}",
                            name=f"{tag}{s}") for s in range(NSET)]

        qT_s = settile("qT", [128, 2, L])
        kT_s = settile("kT", [128, 2, L])
        v_s = settile("v", [128, NK, HPC, 128])
        st_s = settile("st", [128, 2, NQ, 512])
        wq_t = sb.tile([128, NE, DG], cdt, tag="wq", name="wq")
        wk_t = sb.tile([128, NE, DG], cdt, tag="wk", name="wk")
        wv_t = sb.tile([128, NE, DG], cdt, tag="wv", name="wv")
        wo_t = sb.tile([128, 2, E], cdt, tag="wo", name="wo")
        bq_t = sb.tile([128, 2], f32, tag="bq", name="bq")
        bk_t = sb.tile([128, 2], f32, tag="bk", name="bk")
        dm_t = None
        if variant == "causal":
            dm_t = sb.tile([128, 4, 512], f32, tag="dm", name="dm")

        XB = 22

        def w_dma():
            nc.gpsimd.dma_start(out=bq_t, in_=bqs[:, :])
            nc.gpsimd.dma_start(out=bk_t, in_=bks[:, :])
            nc.gpsimd.dma_start(out=wq_t, in_=wq[:, :, :])
            nc.gpsimd.dma_start(out=wk_t, in_=wk[:, :, :])
            nc.gpsimd.dma_start(out=wv_t, in_=wv[:, :, :])
            nc.gpsimd.dma_start(out=wo_t, in_=wo[:, :, :])
            if variant == "causal":
                nc.gpsimd.dma_start(out=dm_t, in_=dmask[:, :, :])

        def x_dma():
            # 48 half-chunk DMAs in projection-consumption order:
            # xk (lp0 ec0-7, lp1 ec0-7), xq same, xv (h0 ec0-7, h1 ec0-7)
            tiles = []
            for xT in (xk, xq, xv):
                for hf in range(2):
                    for ec in range(NE):
                        xt = sb.tile([128, 1, 1024], cdt, tag="xch",
                                     name="xch", bufs=XB)
                        nc.sync.dma_start(
                            out=xt,
                            in_=xT[:, ec:ec + 1, hf * 1024:(hf + 1) * 1024])
                        tiles.append(xt)
            return tiles

        def proj_quanta(s, xtiles, pp):
            # K before Q: what the next phase's attention needs LAST
            # (qT for p=1, V tail) is emitted last.
            for base, w_t, bias_t, scale, outT in (
                (0, wk_t, bk_t, 1.0, kT_s[s]),
                (16, wq_t, bq_t, 0.125, qT_s[s]),
            ):
                for lp in range(2):
                    for dc in range(2):
                        pj = [pp.tile([128, 512], mybir.dt.float32,
                                      tag=f"fq{i}", name=f"fq{i}")
                              for i in range(2)]
                        for ec in range(NE):
                            xt = xtiles[base + lp * NE + ec]
                            for li in range(2):
                                nc.tensor.matmul(
                                    out=pj[li],
                                    lhsT=w_t[:, ec, dc * 128:(dc + 1) * 128],
                                    rhs=xt[:, 0, li * 512:(li + 1) * 512],
                                    start=(ec == 0), stop=(ec == NE - 1),
                                )
                            yield
                        for li in range(2):
                            ln = 2 * lp + li
                            nc.vector.tensor_scalar(
                                out=outT[:, dc, ln * 512:(ln + 1) * 512],
                                in0=pj[li],
                                scalar1=scale,
                                scalar2=bias_t[:, dc:dc + 1],
                                op0=mybir.AluOpType.mult,
                                op1=mybir.AluOpType.add,
                            )
            # V passes: pa -> 4 l-tiles, 8 ec yields
            for pa in range(4):
                hf, off = pa // 2, (pa % 2) * 512
                pv = [pp.tile([128, 2, DG], mybir.dt.float32,
                              tag=f"fq{i}", name=f"fq{i}") for i in range(2)]
                for ec in range(NE):
                    xt = xtiles[32 + hf * NE + ec]
                    for lt4 in range(4):
                        nc.tensor.matmul(
                            out=pv[lt4 // 2][:, lt4 % 2, :],
                            lhsT=xt[:, 0, off + lt4 * 128:off + (lt4 + 1) * 128],
                            rhs=wv_t[:, ec, :],
                            start=(ec == 0 and lt4 % 2 == 0),
                            stop=(ec == NE - 1 and lt4 % 2 == 1),
                        )
                    yield
                for lt4 in range(4):
                    nc.vector.tensor_copy(
                        out=v_s[s][:, 4 * pa + lt4, :, 0:Dh],
                        in_=pv[lt4 // 2][:, lt4 % 2, :]
                        .rearrange("p (h d) -> p h d", h=HPC),
                    )

        def yo_quanta(s, fp):
            # out-projection of set s as 2-bank quanta; yield per jc
            for ln in range(NQ):
                yt = sb.tile([128, NE, 512], odt, tag="yt", name="yt",
                             bufs=2)
                for jc in range(NE):
                    yps = fp.tile([128, 512], mybir.dt.float32,
                                  tag=f"fq{jc % 2}", name=f"fq{jc % 2}")
                    for dc in range(2):
                        nc.tensor.matmul(
                            out=yps,
                            lhsT=wo_t[:, dc, jc * 128:(jc + 1) * 128],
                            rhs=st_s[s][:, dc, ln, :],
                            start=(dc == 0), stop=(dc == 1),
                        )
                    nc.vector.tensor_copy(out=yt[:, jc, :], in_=yps)
                    yield
                nc.gpsimd.dma_start(out=yD[:, ln, :, :], in_=yt)

        def memset_ones():
            for s in range(NSET):
                nc.vector.memset(v_s[s][:, :, :, Dh:128], 1.0)

        def attn_out(s, fill=None, nyield=128):
            from contextlib import nullcontext
            fcm = (tc.tile_pool(name="ps_fill", bufs=1, space="PSUM")
                   if fill else nullcontext())
            with tc.tile_pool(name="ps_att", bufs=2, space="PSUM") as sp, \
                 tc.tile_pool(name="ps_out", bufs=1, space="PSUM") as op, \
                 fcm as fp:
                gen = iter(fill(fp)) if fill else None
                taken, steps_done = 0, 0
                for p in range(2):
                    dc = p
                    for qn in range(NQ):
                        kmax = (min(NK, 4 * qn + 4)
                                if variant == "causal" else NK)
                        po = op.tile([128, 2, 512], mybir.dt.float32,
                                     tag="po", name="po")
                        for kc in range(kmax):
                            db = kc - 4 * qn
                            diag = variant == "causal" and db >= 0
                            qo = db * 128 if diag else 0
                            ps = sp.tile([128, 2, 512], mybir.dt.float32,
                                         tag="pss", name="pss")
                            for hh in range(2):
                                nc.tensor.matmul(
                                    out=ps[:, hh, qo:512],
                                    lhsT=kT_s[s][64 * hh:64 * hh + 64, dc,
                                                 kc * 128:(kc + 1) * 128],
                                    rhs=qT_s[s][64 * hh:64 * hh + 64, dc,
                                                qn * 512 + qo:(qn + 1) * 512],
                                    start=True, stop=True,
                                )
                            mkt = None
                            if variant == "general":
                                mkt = sb.tile([128, 512], mybir.dt.float32,
                                              tag="mkt", name="mkt", bufs=3)
                                nc.sync.dma_start(
                                    out=mkt,
                                    in_=maskT[kc * 128:(kc + 1) * 128,
                                              qn * 512:(qn + 1) * 512])
                                for hh in range(2):
                                    nc.vector.tensor_add(
                                        out=ps[:, hh, :], in0=ps[:, hh, :],
                                        in1=mkt)
                            elif diag:
                                for hh in range(2):
                                    nc.vector.tensor_add(
                                        out=ps[:, hh, qo:qo + 128],
                                        in0=ps[:, hh, qo:qo + 128],
                                        in1=dm_t[:, db, qo:qo + 128])
                            pt = sb.tile([128, 2, 512], cdt, tag="pt",
                                         name="pt", bufs=6)
                            # full contiguous tile (HW-proven AP): for
                            # diag tiles cols 0:qo hold stale scores whose
                            # exp is finite garbage that PV never reads
                            nc.scalar.activation(out=pt, in_=ps, func=Exp)
                            if gen is not None:
                                steps_done += 1
                                want = (nyield * steps_done) // NSTEP
                                while taken < want and \
                                        next(gen, "END") != "END":
                                    taken += 1
                            for hh in range(2):
                                nc.tensor.matmul(
                                    out=po[:, hh, qo:512],
                                    lhsT=v_s[s][:, kc, 2 * p + hh, :],
                                    rhs=pt[:, hh, qo:512],
                                    start=(kc == 0),
                                    stop=(kc == kmax - 1),
                                )
                        rec = sb.tile([64, 2, 512], mybir.dt.float32,
                                      tag="rec", name="rec", bufs=2)
                        for hh in range(2):
                            nc.vector.reciprocal(out=rec[:, hh, :],
                                                 in_=po[64:128, hh, :])
                        for hh in range(2):
                            nc.vector.tensor_mul(
                                out=st_s[s][64 * hh:64 * hh + Dh, dc, qn, :],
                                in0=po[0:Dh, hh, :], in1=rec[:, hh, :])
                if gen is not None:
                    for _ in gen:
                        pass

        def drain(gen_fn):
            with tc.tile_pool(name="ps_drain", bufs=1, space="PSUM") as fp:
                for _ in gen_fn(fp):
                    pass

        def preamble():
            w_dma()
            memset_ones()
            xt0 = x_dma()
            drain(lambda fp: proj_quanta(0, xt0, fp))
            if dual:
                nc.vector.memset(st_s[1][:, :, :, :], 0.0)

        def phase(s):
            xt = x_dma()
            s1 = 1 - s

            def fill(fp):
                yield from yo_quanta(s1, fp)
                yield from proj_quanta(s1, xt, fp)
            attn_out(s, fill=fill, nyield=128)

        if dual and not unroll_dual:
            preamble()
            if reps >= 8:
                with tc.For_i(0, reps // 4, 1, hint_engines=(
                        mybir.EngineType.PE,)):
                    phase(0)
                    phase(1)
                    phase(0)
                    phase(1)
            else:
                with tc.For_i(0, reps // 2, 1, hint_engines=(
                        mybir.EngineType.PE,)):
                    phase(0)
                    phase(1)
            drain(lambda fp: yo_quanta(1, fp))
        elif dual:
            preamble()
            for _ in range(reps // 2):
                phase(0)
                phase(1)
            drain(lambda fp: yo_quanta(1, fp))
        else:
            for _ in range(reps):
                preamble()
                attn_out(0)
                drain(lambda fp: yo_quanta(0, fp))

    nc.finalize()
    return nc


def _get_nc(variant, dt_mode=None, reps=1, hw_loop=False, unroll_dual=False):
    dt_mode = dt_mode or DT_MODE
    key = (variant, dt_mode, reps, hw_loop, unroll_dual)
    if key not in _CACHE:
        _CACHE[key] = _build(variant, dt_mode, reps, hw_loop, unroll_dual)
    return _CACHE[key]


def _detect_variant(mask):
    m2 = np.asarray(mask).reshape(mask.shape[-2], mask.shape[-1])
    m01 = (m2 != 0)
    if m01.all():
        return "none", m2
    if np.array_equal(m01, np.tril(np.ones(m2.shape, bool))):
        return "causal", m2
    return "general", m2


def _dmask_np():
    kl = np.arange(128)[:, None, None]
    db = np.arange(4)[None, :, None]
    ql = np.arange(512)[None, None, :]
    return np.where(db * 128 + kl > ql, NEG, np.float32(0)).astype(np.float32)


def _pack_pm(aT):
    # [R, C] with R = NE*128 -> [128, NE, C] partition-major
    r, c = aT.shape
    return np.ascontiguousarray(
        aT.reshape(r // 128, 128, c).transpose(1, 0, 2))


def _cvt(a, dt_mode):
    import ml_dtypes
    return np.ascontiguousarray(a.astype(ml_dtypes.bfloat16))


def _make_in_maps(x_q, x_k, x_v, m2, variant, Wq, bq, Wk, bk, Wv, Wo,
                  dt_mode=None):
    dt_mode = dt_mode or DT_MODE
    in_maps = []
    madd = None
    if variant == "general":
        madd = np.ascontiguousarray(
            np.where(m2 == 0, NEG, np.float32(0)).astype(np.float32).T)
    dmn = _dmask_np() if variant == "causal" else None
    xs = {b: tuple(_cvt(_pack_pm(x[b].T), dt_mode)
                   for x in (x_q, x_k, x_v)) for b in range(B)}
    for c in range(NCORES):
        b, g = divmod(c, HPC)
        gs = slice(g * DG, (g + 1) * DG)
        # wo2[(h%2)*64 + d, h//2, :] = Wo[:, g*DG + h*64 + d]
        wog = Wo[:, gs].T.reshape(HPC, Dh, E)
        wo2 = np.zeros((128, 2, E), np.float32)
        for h in range(HPC):
            wo2[(h % 2) * Dh:(h % 2 + 1) * Dh, h // 2, :] = wog[h]
        im = {
            "xq": xs[b][0],
            "xk": xs[b][1],
            "xv": xs[b][2],
            "wq": _cvt(_pack_pm(Wq[gs, :].T), dt_mode),
            "wk": _cvt(_pack_pm(Wk[gs, :].T), dt_mode),
            "wv": _cvt(_pack_pm(Wv[gs, :].T), dt_mode),
            "bqs": np.ascontiguousarray((bq[gs] / 8.0).reshape(2, 128).T),
            "bks": np.ascontiguousarray(bk[gs].reshape(2, 128).T),
            "wo": _cvt(wo2, dt_mode),
        }
        if variant == "causal":
            im["dmask"] = dmn
        elif variant == "general":
            im["maskT"] = madd
        in_maps.append(im)
    return in_maps


def kernel(x_q, x_k, x_v, mask, Wq, bq, Wk, bk, Wv, bv, Wo, bo):
    global LAST_RESULTS
    from concourse.bass_utils import run_bass_kernel_spmd

    x_q = np.asarray(x_q, np.float32)
    x_k = np.asarray(x_k, np.float32)
    x_v = np.asarray(x_v, np.float32)
    Wq = np.asarray(Wq, np.float32)
    Wk = np.asarray(Wk, np.float32)
    Wv = np.asarray(Wv, np.float32)
    Wo = np.asarray(Wo, np.float32)
    bq = np.asarray(bq, np.float32)
    bk = np.asarray(bk, np.float32)
    bv = np.asarray(bv, np.float32)
    bo = np.asarray(bo, np.float32)

    variant, m2 = _detect_variant(mask)
    nc = _get_nc(variant)
    in_maps = _make_in_maps(x_q, x_k, x_v, m2, variant, Wq, bq, Wk, bk, Wv, Wo)

    res = run_bass_kernel_spmd(nc, in_maps, core_ids=list(range(NCORES)))
    LAST_RESULTS = res

    corr = (bo + Wo @ bv).astype(np.float32)
    y = np.empty((B, L, E), np.float32)
    for b in range(B):
        acc = res.results[HPC * b]["yD"].astype(np.float32)
        for g in range(1, HPC):
            acc += res.results[HPC * b + g]["yD"].astype(np.float32)
        # [128(p), NQ(ln), NE(jc), 512(q)] -> [L, E]
        y[b] = acc.transpose(1, 3, 2, 0).reshape(L, E) + corr
    return y
